# revision 3
# baseline (speedup 1.0000x reference)
"""Trainium2 Bass kernel for nn_Attention_44564580663760 — v2.

Single-head "attention" (B=8, S=2048, D=1024, fp32) with the reference's
quirk reproduced: scores = q @ v^T (k projection unused), causal mask,
softmax, ctx @ v, output projection.

Sharding: data-parallel — one batch element per NeuronCore (8 cores).

v2 changes vs baseline:
  - qT and vT stay resident in SBUF (no DRAM round-trips).
  - v natural layout built with PE transposes (no DMA transposes).
  - softmax denominators: DVE-accumulated E_sum + one tiny matmul per
    q-tile (replaces 160 N=1 matmuls).
  - attention processed in 1024-wide macro blocks (2 per core): halves
    the ldweights count in scores/ctx.
  - accumulation ordered so consumers never stall the PE (ctx d-order
    feeds out-proj in completion order).

Per-core dataflow (matmul contracts the partition dim):
    xT  [d, s]  (host-transposed input, bf16)
    qT  [e, s] = WqT.T @ xT (+bq)   resident
    vT  [e, s] = WvT.T @ xT (+bv)   resident
    v   [k, kt, d] = PE-transpose(vT) resident ("vbig")
    per macro block (1024 q cols):
      scoresT [k, q] = vT.T @ qT    (fp32 psum, 2x 512-col chunks)
      eT = exp(scoresT/32) (bf16), causal-masked; E_sum += eT (DVE fp32)
      ctxT [d, q]: lhsT = vbig slices, rhs = eT     (ragged on diag)
      l[q] = E_sum.T @ ones  (one N=1 matmul per 128-q tile)
      out [q, e]: lhsT = ctxT slices, rhs = WoT; scaled by 1/l, + bo
"""

import sys

sys.path.insert(0, "/opt/trn_rl_repo")

import contextlib

import numpy as np

import concourse.bacc as bacc
import concourse.bass as bass
import concourse.mybir as mybir
import concourse.tile as tile
from concourse.bass_utils import run_bass_kernel_spmd
from concourse.masks import make_identity

FP32 = mybir.dt.float32
BF16 = mybir.dt.bfloat16

B, S, D = 8, 2048, 1024
PT = 128
NTS = S // PT  # 16 s-tiles
NTD = D // PT  # 8 d-tiles
MB = 1024  # macro q-block width
NMB = S // MB  # 2
KPM = MB // PT  # 8 k-tiles per macro block
SCALE = 1.0 / np.sqrt(np.float32(D))
ACT = mybir.ActivationFunctionType


def build_nc(causal: bool, reps: int = 0, phases: str = "all") -> bass.Bass:
    nc = bacc.Bacc("TRN2", target_bir_lowering=False, debug=False)
    dram = {
        "xT": nc.declare_dram_parameter("xT", [D, S], BF16, isOutput=False),
        "WqT": nc.declare_dram_parameter("WqT", [D, D], BF16, isOutput=False),
        "WvT": nc.declare_dram_parameter("WvT", [D, D], BF16, isOutput=False),
        "WoT": nc.declare_dram_parameter("WoT", [D, D], BF16, isOutput=False),
        "bqc": nc.declare_dram_parameter("bqc", [PT, NTD], FP32, isOutput=False),
        "bvc": nc.declare_dram_parameter("bvc", [PT, NTD], FP32, isOutput=False),
        "bob": nc.declare_dram_parameter("bob", [PT, D], FP32, isOutput=False),
        "utri": nc.declare_dram_parameter("utri", [PT, PT], BF16, isOutput=False),
        "out": nc.declare_dram_parameter("out", [S, D], FP32, isOutput=True),
    }

    with (
        tile.TileContext(nc) as tc,
        tc.tile_pool(name="const", bufs=1) as constp,
        tc.tile_pool(name="qTp", bufs=1) as qTp,
        tc.tile_pool(name="vTp", bufs=1) as vTp,
    ):
        qT_t = [qTp.tile([PT, S], BF16, tag=f"qT{e}", name=f"qT{e}") for e in range(NTD)]
        vT_t = [vTp.tile([PT, S], BF16, tag=f"vT{e}", name=f"vT{e}") for e in range(NTD)]
        with tc.tile_pool(name="dramp", bufs=1, space="DRAM") as dramp:
            loop_ctx = tc.For_i(0, reps, 1) if reps else contextlib.nullcontext()
            with loop_ctx:
                _body(nc, tc, causal, constp, dram, qT_t, vT_t, dramp, phases)
    _dedup_ldweights(nc)
    nc.finalize()
    return nc


def _dedup_ldweights(nc):
    """Drop InstLdweights whose stationary operand matches the previous PE
    weight load (no intervening PE weight change) — the paired matmuls then
    reuse the already-loaded weights. Deps of a dropped LDW move to the next
    kept instruction so semaphore generation still orders correctly."""
    removed = {}
    n_drop = 0
    for bb in nc.main_func.blocks:
        insts = bb.instructions
        keep = []
        last_sig = None
        pending = []
        for ins in insts:
            drop = False
            if isinstance(ins, mybir.InstLdweights):
                sig = (
                    str(ins.ins[0]),
                    bool(ins.is_transpose),
                    str(ins.perf_mode),
                    str(ins.tile_position),
                )
                if sig == last_sig:
                    drop = True
                else:
                    last_sig = sig
            elif (
                getattr(ins, "engine", None) == mybir.EngineType.PE
                and isinstance(ins, mybir.InstMatmult)
                and ins.is_transpose
            ):
                # transpose-mode matmuls change the loaded weights
                last_sig = None
            if drop:
                pending.append(ins)
                n_drop += 1
                continue
            for p in pending:
                ins.merge_dependencies_from(p)
                removed[p.name] = ins.name
            pending = []
            keep.append(ins)
        assert not pending
        if len(keep) != len(insts):
            insts[:] = keep
    if removed:
        for bb in nc.main_func.blocks:
            for ins in bb.instructions:
                ins.remap_dependency_names(removed)
        if hasattr(nc, "inst_map"):
            for name in removed:
                nc.inst_map.pop(name, None)


def _projections(nc, tc, constp, dram, qT_t, vT_t):
    with (
        tc.tile_pool(name="xw", bufs=1) as xwp,
        tc.tile_pool(name="psA", bufs=2, space="PSUM") as psAp,
    ):
        # batched bias loads first (an activation stalled on its bias
        # blocks the psum rotation; one DMA each, [128, 8] e-column layout)
        bva = constp.tile([PT, NTD], FP32, tag="bva", name="bva")
        nc.sync.dma_start(bva[:], dram["bvc"][:, :])
        bqa = constp.tile([PT, NTD], FP32, tag="bqa", name="bqa")
        nc.sync.dma_start(bqa[:], dram["bqc"][:, :])
        bv_t = [bva[:, e : e + 1] for e in range(NTD)]
        bq_t = [bqa[:, e : e + 1] for e in range(NTD)]
        xT_t, Wv_t, Wq_t = [], [], []
        for k in range(NTD):
            xt = xwp.tile([PT, S], BF16, tag=f"xT{k}", name=f"xT{k}")
            nc.sync.dma_start(xt[:], dram["xT"][k * PT : (k + 1) * PT, :])
            xT_t.append(xt)
            wv = xwp.tile([PT, D], BF16, tag=f"wv{k}", name=f"wv{k}")
            nc.sync.dma_start(wv[:], dram["WvT"][k * PT : (k + 1) * PT, :])
            Wv_t.append(wv)
        for k in range(NTD):
            wq = xwp.tile([PT, D], BF16, tag=f"wq{k}", name=f"wq{k}")
            nc.sync.dma_start(wq[:], dram["WqT"][k * PT : (k + 1) * PT, :])
            Wq_t.append(wq)

        for which in ("v", "q"):
            for e in range(NTD):
                W_t = Wv_t if which == "v" else Wq_t
                b_t = bv_t if which == "v" else bq_t
                dst = vT_t[e] if which == "v" else qT_t[e]
                # sc-outer k-inner accumulation; act per finished 512-chunk
                # so the activation pipelines under the next chunk's matmuls
                ps = psAp.tile([PT, S], FP32, tag="ps", name="ps")
                for sc in range(S // 512):
                    c0, c1 = sc * 512, (sc + 1) * 512
                    for k in range(NTD):
                        nc.tensor.matmul(
                            ps[:, c0:c1],
                            W_t[k][:, e * PT : (e + 1) * PT],
                            xT_t[k][:, c0:c1],
                            start=(k == 0),
                            stop=(k == NTD - 1),
                        )
                    nc.scalar.activation(
                        dst[:, c0:c1], ps[:, c0:c1], ACT.Identity,
                        bias=b_t[e], scale=1.0,
                    )



def _body(nc, tc, causal, constp, dram, qT_t, vT_t, dramp, phases="all"):
    # phases: "all" or prefix subset for HW phase attribution:
    #   "proj" | "proj+tp" | "proj+tp+scores" | "proj+tp+scores+ctx"
    _projections(nc, tc, constp, dram, qT_t, vT_t)
    do_tp = phases == "all" or "+tp" in phases
    do_scores = phases == "all" or "+scores" in phases
    do_ctx = phases == "all" or "+ctx" in phases
    do_out = phases == "all"
    if not do_tp:
        # keep the out store so nothing is dead-code-eliminated
        with tc.tile_pool(name="stub", bufs=1) as stp:
            sb = stp.tile([PT, D], FP32, tag="sb", name="sb")
            nc.vector.tensor_copy(sb[:, 0:1], vT_t[0][:, 0:1])
            nc.sync.dma_start(dram["out"][0:PT, :], sb[:])
        return

    # constants for attention (emitted late so they don't delay xT/W loads)
    ident = constp.tile([PT, PT], BF16, tag="ident", name="ident")
    make_identity(nc, ident[:])
    WoT_t = []
    for d in range(NTD):
        w = constp.tile([PT, D], BF16, tag=f"wot{d}", name=f"wot{d}")
        nc.sync.dma_start(w[:], dram["WoT"][d * PT : (d + 1) * PT, :])
        WoT_t.append(w)
    bo_t = constp.tile([PT, D], FP32, tag="bo", name="bo")
    nc.sync.dma_start(bo_t[:], dram["bob"][:, :])
    utri_t = constp.tile([PT, PT], BF16, tag="utri", name="utri")
    nc.sync.dma_start(utri_t[:], dram["utri"][:, :])
    ones_f = constp.tile([PT, 1], FP32, tag="onesf", name="onesf")
    nc.gpsimd.memset(ones_f[:], 1.0)

    with tc.tile_pool(name="vb", bufs=1) as vbp:
        vbig = vbp.tile([PT, NTS, D], BF16, tag="vbig", name="vbig")

        # ---- v natural: PE-transpose vT -> vbig [k, kt, d] ----
        # zero DRAM traffic: the 8-core graded run is HBM-bandwidth
        # sensitive, so 13.7us of PE beats an 8MB DRAM round-trip
        with tc.tile_pool(name="tp", bufs=2, space="PSUM") as tpp:
            for e in range(NTD):
                tps = tpp.tile([PT, NTS, PT], BF16, tag="tp", name="tp")
                for k in range(NTS):
                    nc.tensor.transpose(
                        tps[:, k, :], vT_t[e][:, k * PT : (k + 1) * PT], ident[:]
                    )
                nc.vector.tensor_copy(vbig[:, :, e * PT : (e + 1) * PT], tps[:])

        if not do_scores:
            with tc.tile_pool(name="stub", bufs=1) as stp:
                sb = stp.tile([PT, D], BF16, tag="sb", name="sb")
                nc.vector.tensor_copy(sb[:], vbig[:, 0, :])
                nc.sync.dma_start(
                    dram["out"][0:PT, 0 : D // 2], sb[:].bitcast(FP32)
                )
            return

        # ---- attention macro blocks ----
        with (
            tc.tile_pool(name="eT", bufs=1) as eTp,
            tc.tile_pool(name="es", bufs=1) as esp,
            tc.tile_pool(name="ctx", bufs=1) as ctxp,
            tc.tile_pool(name="rl", bufs=1) as rlp,
            tc.tile_pool(name="ost", bufs=2) as ostp,
            tc.tile_pool(name="psP", bufs=3, space="PSUM") as psPp,
            tc.tile_pool(name="psC", bufs=2, space="PSUM") as psCp,
            tc.tile_pool(name="psL", bufs=1, space="PSUM") as psLp,
        ):
            eT_t = [
                eTp.tile([PT, MB], BF16, tag=f"e{ki}", name=f"e{ki}")
                for ki in range(NTS)
            ]
            psl = psLp.tile([PT, NTS], FP32, tag="l", name="l")
            rl_t = rlp.tile([PT, NTS], FP32, tag="rl", name="rl")

            for m in range(NMB):
                q0 = m * MB
                kmax = KPM * (m + 1) if causal else NTS
                esum = esp.tile([PT, MB], FP32, tag="es", name="es")

                # scores + exp + E_sum
                for ki in range(kmax):
                    mp = ki - KPM * m  # diag sub-index if >= 0
                    lo = mp * PT if (causal and mp > 0) else 0
                    ps_c = []
                    for ch in range(2):
                        c0, c1 = ch * 512, (ch + 1) * 512
                        if lo >= c1:
                            ps_c.append(None)
                            continue
                        pch = psPp.tile([PT, 512], FP32, tag="ps", name="ps")
                        ps_c.append(pch)
                    for ch in range(2):
                        if ps_c[ch] is None:
                            continue
                        c0 = ch * 512
                        l0 = max(lo - c0, 0)
                        for e in range(NTD):
                            nc.tensor.matmul(
                                ps_c[ch][:, l0:512],
                                vT_t[e][:, ki * PT : (ki + 1) * PT],
                                qT_t[e][:, q0 + c0 + l0 : q0 + c0 + 512],
                                start=(e == 0),
                                stop=(e == NTD - 1),
                            )
                    et = eT_t[ki]
                    if causal and mp > 0:
                        nc.gpsimd.memset(et[:, 0:lo], 0.0)
                    for ch in range(2):
                        if ps_c[ch] is None:
                            continue
                        c0 = ch * 512
                        l0 = max(lo - c0, 0)
                        nc.scalar.activation(
                            et[:, c0 + l0 : c0 + 512],
                            ps_c[ch][:, l0:512],
                            ACT.Exp,
                            scale=float(SCALE),
                        )
                    if causal and mp >= 0:
                        nc.vector.tensor_mul(
                            et[:, mp * PT : (mp + 1) * PT],
                            et[:, mp * PT : (mp + 1) * PT],
                            utri_t[:],
                        )
                    if ki == 0:
                        nc.vector.tensor_copy(esum[:], et[:])
                    else:
                        nc.vector.tensor_add(esum[:], esum[:], et[:])

                # softmax denominators: l = E_sum.T @ ones, rl = 1/l
                for qt in range(KPM):
                    qtg = m * KPM + qt
                    nc.tensor.matmul(
                        psl[:, qtg : qtg + 1],
                        esum[:, qt * PT : (qt + 1) * PT],
                        ones_f[:],
                        start=True,
                        stop=True,
                    )
                nc.vector.reciprocal(
                    rl_t[:, m * KPM : (m + 1) * KPM],
                    psl[:, m * KPM : (m + 1) * KPM],
                )
                if not do_ctx:
                    nc.sync.dma_start(
                        dram["out"][m * PT : (m + 1) * PT, 0:NTS], rl_t[:]
                    )
                    continue

                # ctxT[d, q], accumulated over ki (ragged on diag)
                for d in range(NTD):
                    pc = psCp.tile([PT, MB], FP32, tag="c", name="c")
                    for ch in range(2):
                        c0, c1 = ch * 512, (ch + 1) * 512
                        # ki whose causal column range reaches this chunk
                        kis = []
                        for ki in range(kmax):
                            mp = ki - KPM * m
                            lo = mp * PT if (causal and mp > 0) else 0
                            if lo < c1:
                                kis.append((ki, max(lo - c0, 0)))
                        for j, (ki, l0) in enumerate(kis):
                            nc.tensor.matmul(
                                pc[:, c0 + l0 : c1],
                                vbig[:, ki, d * PT : (d + 1) * PT],
                                eT_t[ki][:, c0 + l0 : c1],
                                start=(j == 0),
                                stop=(j == len(kis) - 1),
                                skip_group_check=True,
                            )
                    cx = ctxp.tile([PT, MB], BF16, tag=f"cx{d}", name=f"cx{d}")
                    nc.vector.tensor_copy(cx[:], pc[:])
                    if d == 0:
                        ctx_t = []
                    ctx_t.append(cx)

                if not do_out:
                    nc.sync.dma_start(
                        dram["out"][q0 : q0 + PT, 0 : MB // 2],
                        ctx_t[0][:, 0:MB].bitcast(FP32),
                    )
                    continue

                # out projection + normalize + bias + store
                for qt in range(KPM):
                    qtg = m * KPM + qt
                    os_ = ostp.tile([PT, D], FP32, tag="os", name="os")
                    # last tile of the kernel: finer chunks shorten the tail
                    nch = 4 if (m == NMB - 1 and qt == KPM - 1) else 1
                    w = 512 // nch
                    for ec in range(2):
                        po = psPp.tile([PT, 512], FP32, tag="ps", name="ps")
                        for d in range(NTD):
                            nc.tensor.matmul(
                                po[:],
                                ctx_t[d][:, qt * PT : (qt + 1) * PT],
                                WoT_t[d][:, ec * 512 : (ec + 1) * 512],
                                start=(d == 0),
                                stop=(d == NTD - 1),
                            )
                        for h in range(nch):
                            c0 = ec * 512 + h * w
                            nc.vector.scalar_tensor_tensor(
                                os_[:, c0 : c0 + w],
                                po[:, h * w : (h + 1) * w],
                                rl_t[:, qtg : qtg + 1],
                                bo_t[:, c0 : c0 + w],
                                mybir.AluOpType.mult,
                                mybir.AluOpType.add,
                            )
                            nc.sync.dma_start(
                                dram["out"][
                                    q0 + qt * PT : q0 + (qt + 1) * PT, c0 : c0 + w
                                ],
                                os_[:, c0 : c0 + w],
                            )


_TRIL = None


def _detect_causal(mask: np.ndarray) -> bool:
    global _TRIL
    m0 = np.asarray(mask[0])
    if bool(m0[0, 1]):
        if not m0.all() or not np.asarray(mask).all():
            raise NotImplementedError("unsupported mask pattern")
        return False
    if _TRIL is None:
        _TRIL = np.tril(np.ones((S, S), dtype=bool))
    for b in range(mask.shape[0]):
        if not np.array_equal(np.asarray(mask[b]), _TRIL):
            raise NotImplementedError("unsupported mask pattern")
    return True


def _base_inputs(Wq, bq, Wv, bv, Wo, bo):
    import ml_dtypes

    WqT = np.ascontiguousarray(np.asarray(Wq, dtype=np.float32).T).astype(
        ml_dtypes.bfloat16
    )
    WvT = np.ascontiguousarray(np.asarray(Wv, dtype=np.float32).T).astype(
        ml_dtypes.bfloat16
    )
    WoT = np.ascontiguousarray(np.asarray(Wo, dtype=np.float32).T).astype(
        ml_dtypes.bfloat16
    )
    return {
        "WqT": WqT,
        "WvT": WvT,
        "WoT": WoT,
        "bqc": np.ascontiguousarray(
            np.asarray(bq, dtype=np.float32).reshape(NTD, PT).T
        ),
        "bvc": np.ascontiguousarray(
            np.asarray(bv, dtype=np.float32).reshape(NTD, PT).T
        ),
        "bob": np.tile(np.asarray(bo, dtype=np.float32).reshape(1, D), (PT, 1)),
        "utri": np.triu(np.ones((PT, PT), dtype=np.float32)).astype(
            ml_dtypes.bfloat16
        ),
    }


def make_inputs_for_bench(rng):
    import ml_dtypes

    x = rng.standard_normal((S, D), dtype=np.float32)
    mk = lambda *s: (rng.standard_normal(s, dtype=np.float32) * 0.02)
    base = _base_inputs(mk(D, D), mk(D), mk(D, D), mk(D), mk(D, D), mk(D))
    base["xT"] = np.ascontiguousarray(x.T).astype(ml_dtypes.bfloat16)
    return base


def kernel(x, mask, Wq, bq, Wk, bk, Wv, bv, Wo, bo):
    import ml_dtypes

    x = np.asarray(x, dtype=np.float32)
    causal = _detect_causal(np.asarray(mask))
    nc = build_nc(causal)
    base = _base_inputs(Wq, bq, Wv, bv, Wo, bo)
    in_maps = [
        {"xT": np.ascontiguousarray(x[b].T).astype(ml_dtypes.bfloat16), **base}
        for b in range(B)
    ]
    res = run_bass_kernel_spmd(nc, in_maps, list(range(B)))
    out = np.stack([np.asarray(res.results[i]["out"]) for i in range(B)])
    return out.astype(np.float32)


if __name__ == "__main__":
    rng = np.random.default_rng(0)
    x = rng.standard_normal((B, S, D), dtype=np.float32)
    mask = np.broadcast_to(np.tril(np.ones((S, S), dtype=bool)), (B, S, S))
    mk = lambda *s: (rng.standard_normal(s, dtype=np.float32) * 0.02)
    out = kernel(
        x, mask, mk(D, D), mk(D), mk(D, D), mk(D), mk(D, D), mk(D), mk(D, D), mk(D)
    )
    print(out.shape, out.dtype)
